# revision 22
# baseline (speedup 1.0000x reference)
"""Trainium2 Bass kernel for nn_DiffusionDynamicInput.

Reference computation (per sample b):
    ctx  = wv_embs[b] + t_emb[b]                       (13, 1024)
    hid  = silu(ctx @ w1 + b1)                         (13, 512)
    wgen = (hid @ w2 + b2).reshape(13, 128, 9)         per-(band) 3x3 filters
    out[d,h,w] = sum_{n,dy,dx} wgen[n,d,(dy,dx)] * x[b,n,h+dy,w+dx]   (SAME pad)
    bias = (ctx @ wb + bb).sum(axis=0)                 (128,)
    out += bias[:, None, None]

Sharding: data-parallel over B=8 across the 8 NeuronCores (one sample per
core).

Dynamic conv: ALL 117 taps (13 bands x 9 filter positions) are packed into
the matmul contraction dim, one matmul per two output rows: partition
q = 1 + n*9 + (dyi*3 + dxi) holds the image of band n shifted by (dy, dx);
both shifts are materialized at build time by 9 strided DMA loads from the
host-padded fp16 image (the dx shift is a column offset into the 258-wide
padded rows, the dy shift a row offset). Partition 0 is a constant-ones
row so the per-sample bias (stored as lhsT partition 0) is added by the
same matmul. PSUM holds fp32; evictions convert to fp16 and are split
across the DVE/ACT engines (GPSIMD cannot read PSUM); the output is
written as fp16 (halving the 33.5 MB/core fp32 write that bounded the
previous version) and upcast to fp32 on the host. Each 16-row staging
tile is written out as two 8-row DMAs (SP HWDGE ring + GPSIMD SWDGE) so
each transfer starts as soon as its half of the evictions lands; the
evicting engines never trigger DMAs (a dma_start on their in-order SEQ
would stall later evictions on the other engine's semaphores).
"""

import numpy as np

import concourse.bacc as bacc
import concourse.mybir as mybir
import concourse.tile as tile
from concourse.bass_utils import run_bass_kernel_spmd
from concourse.masks import make_identity

F32 = mybir.dt.float32
F16 = mybir.dt.float16

NB = 13          # bands
HH = WW = 256    # image
DE = 1024        # embed dim
DO = 128         # out channels
NCORES = 8

WPAD = WW + 2    # 258: host-padded row layout (zero column at each end)
KQ = NB * 9 + 1  # contraction partitions: ones row + 117 shifted images
GRP_ROWS = 16    # output rows per staging tile / output DMA (8 KB/partition)
NGRP = HH // GRP_ROWS


def _build_bass(repeat: int = 1, ablate: str = ""):
    # Bacc (not plain Bass): its finalize() runs generate_event_semaphores,
    # which splits multi-sem waits that TRN2 instruction structs can't hold.
    # repeat > 1 re-emits the main conv loop (benchmarking: slope between
    # repeat counts isolates device time from dispatch overhead).
    ab = set(ablate.split(",")) if ablate else set()
    nc = bacc.Bacc(target_bir_lowering=False, debug=False)

    # x is host-cast to fp16 and host-padded to 258-wide rows (zero col at
    # each end), so the 9 shifted-image DMAs are strided but dense reads
    x_ext = nc.declare_dram_parameter("x", [NB, HH, WPAD], F16, isOutput=False)
    t_ext = nc.declare_dram_parameter("t_emb", [DE], F32, isOutput=False)
    wv_ext = nc.declare_dram_parameter("wv", [NB, DE], F32, isOutput=False)
    # w1/w2p/wb are host-cast to fp16; w2p/b2p host-permuted so generated
    # filter column c' = p*128 + d
    # w1p[p, k, m*128+s] = w1[k*128+p, m*128+s]; similarly w2p along k;
    # wbp[p, k, d] = wb[k*128+p, d]  (one contiguous DMA per weight)
    w1_ext = nc.declare_dram_parameter("w1p", [128, 8, 4 * DO], F16, isOutput=False)
    b1_ext = nc.declare_dram_parameter("b1", [4 * DO], F32, isOutput=False)
    w2p_ext = nc.declare_dram_parameter("w2pp", [128, 4, DO * 9], F16, isOutput=False)
    b2p_ext = nc.declare_dram_parameter("b2p", [DO * 9], F16, isOutput=False)
    wb_ext = nc.declare_dram_parameter("wbp", [128, 8, DO], F16, isOutput=False)
    bb13_ext = nc.declare_dram_parameter("bb13", [DO], F16, isOutput=False)
    ones_ext = nc.declare_dram_parameter("ones", [1, HH, WW], F16, isOutput=False)
    out_ext = nc.declare_dram_parameter("out", [DO, HH, WW], F16, isOutput=True)

    with tile.TileContext(nc) as tc:
        with (
            tc.tile_pool(name="const", bufs=1) as const_pool,
            tc.tile_pool(name="resident", bufs=1) as res_pool,
        ):
            # ------- phase 0: build the 117 shifted fp16 images in SBUF -----
            # x117[1 + n*9 + dyi*3 + dxi, r, c] = x[n, r+dy, c+dx]  (zeros at
            # the image edge); partition 0 = 1.0 everywhere (bias row — at
            # partition 0 because compute engines need aligned partition
            # bases, while the tap DMAs can start anywhere).
            x117 = res_pool.tile([KQ, HH, WW], F16)
            # rows no DMA writes (image edge): zero across all partitions
            # first; the ones DMA and in-range dy groups' DMAs overwrite.
            nc.gpsimd.memset(x117[:, 0:1, :], 0.0)
            nc.gpsimd.memset(x117[:, HH - 1:HH, :], 0.0)
            nc.sync.dma_start(x117[0:1, :, :], ones_ext.ap())
            x117_g = x117[1:KQ, :, :].rearrange("(n t) r c -> n t r c", t=9)
            for dyi, dy in enumerate((-1, 0, 1)):
                lo = max(0, -dy)
                hi = min(HH, HH - dy)
                for dxi in range(3):
                    t = dyi * 3 + dxi
                    nc.sync.dma_start(
                        x117_g[:, t, lo:hi, :],
                        x_ext.ap()[:, lo + dy:hi + dy, dxi:dxi + WW],
                    )

            # lhsT[1 + n*9 + t, d] = wgen16[n, t*128 + d]; partition 0 = bias
            # (matches the x117 ones row). Lives in the resident pool so the
            # hypernetwork pool can be released before the conv loop.
            lhsT = res_pool.tile([KQ, DO], F16)

            ident = const_pool.tile([128, 128], F32)
            make_identity(nc, ident[:])
            ones1 = const_pool.tile([1, NB], F16)
            nc.vector.memset(ones1[:], 1.0)
            ones11 = const_pool.tile([1, 1], F16)
            nc.vector.memset(ones11[:], 1.0)

            # ---------------- hypernetwork (fp16 in / fp32 psum) ------------
            with tc.tile_pool(name="hyp", bufs=1) as hyp_pool:
                tT = hyp_pool.tile([128, 8], F32)   # t_emb[k*128+p] -> [p, k]
                nc.sync.dma_start(tT[:], t_ext.ap().rearrange("(k p) -> p k", p=128))
                b1T = hyp_pool.tile([128, 4], F32)
                nc.sync.dma_start(b1T[:], b1_ext.ap().rearrange("(m p) -> p m", p=128))
                bb13r = hyp_pool.tile([1, DO], F16)  # 13 * bb as a row
                nc.sync.dma_start(
                    bb13r[:], bb13_ext.ap().rearrange("(o c) -> o c", o=1)
                )
                b2pT = hyp_pool.tile([1, DO * 9], F16)
                nc.sync.dma_start(b2pT[:], b2p_ext.ap().rearrange("(o c) -> o c", o=1))

                wv_t = hyp_pool.tile([NB, DE], F32)
                nc.sync.dma_start(wv_t[:], wv_ext.ap())

                w1p_t = hyp_pool.tile([128, 8, 4 * DO], F16)
                nc.sync.dma_start(w1p_t[:], w1_ext.ap())
                w2p_t = hyp_pool.tile([128, 4, DO * 9], F16)
                nc.sync.dma_start(w2p_t[:], w2p_ext.ap())
                wbp_t = hyp_pool.tile([128, 8, DO], F16)
                nc.sync.dma_start(wbp_t[:], wb_ext.ap())

                # ctxT[e, k, n] = wv[n, k*128+e] + t[k*128+e]   (fp16)
                ctxT = hyp_pool.tile([128, 8, NB], F16)
                with tc.tile_pool(name="tp_psum", bufs=2, space="PSUM") as tp_psum:
                    # warm-up op: absorbs the identity-producer (Pool)
                    # semaphore into the PE engine clock so later transposes
                    # carry a single wait (the fused LDW struct has one wait
                    # slot).
                    ps_warm = tp_psum.tile([1, 1], F32, tag="warm", bufs=1)
                    nc.tensor.transpose(ps_warm[:], ident[:1, :1], ident[:1, :1])
                    for k in range(8):
                        ps = tp_psum.tile([128, NB], F32, tag="tp")
                        nc.tensor.transpose(
                            ps[:], wv_t[:, k * 128:(k + 1) * 128], ident[:NB, :NB]
                        )
                        nc.vector.tensor_scalar_add(
                            ctxT[:, k, :], ps[:], tT[:, k:k + 1]
                        )

                    # sT[e, k] = sum_n ctxT[e, k, n]   (fp16 for the wb matmul)
                    sT32 = hyp_pool.tile([128, 8, 1], F32)
                    nc.vector.reduce_sum(sT32[:], ctxT[:], axis=mybir.AxisListType.X)
                    sT = hyp_pool.tile([128, 8, 1], F16)
                    nc.vector.tensor_copy(sT[:], sT32[:])

                    # hidT[s, m, n] = silu(sum_e w1[e, m*128+s] * ctxT[e, n]
                    #                      + b1)
                    hidT = hyp_pool.tile([128, 4, NB], F16)
                    for m in range(4):
                        ps = tp_psum.tile([128, NB], F32, tag="hid")
                        for k in range(8):
                            nc.tensor.matmul(
                                ps[:], w1p_t[:, k, m * 128:(m + 1) * 128],
                                ctxT[:, k, :], start=(k == 0), stop=(k == 7)
                            )
                        nc.scalar.activation(
                            hidT[:, m, :], ps[:],
                            mybir.ActivationFunctionType.Silu, bias=b1T[:, m:m + 1],
                        )

                    # wgen16[n, p*128+d] = hid @ w2p + b2p   (fp16)
                    wgen16 = hyp_pool.tile([NB, DO * 9], F16)
                    for j in range(3):  # 1152 = 3 * 384
                        ps = tp_psum.tile([NB, 384], F32, tag="wgen")
                        for k in range(4):
                            nc.tensor.matmul(
                                ps[:], hidT[:, k, :],
                                w2p_t[:, k, j * 384:(j + 1) * 384],
                                start=(k == 0), stop=False,
                            )
                        nc.tensor.matmul(
                            ps[:], ones1[:], b2pT[:, j * 384:(j + 1) * 384],
                            start=False, stop=True,
                        )
                        nc.vector.tensor_copy(wgen16[:, j * 384:(j + 1) * 384], ps[:])

                    # bias row: bias[d] = sum_e s[e] * wb[e, d] + 13 * bb[d],
                    # computed directly as a [1, 128] psum row (s stationary)
                    ps_b = tp_psum.tile([1, DO], F32, tag="bias", bufs=1)
                    for k in range(8):
                        nc.tensor.matmul(
                            ps_b[:], sT[:, k, :], wbp_t[:, k, :],
                            start=(k == 0), stop=False,
                        )
                    nc.tensor.matmul(
                        ps_b[:], ones11[:], bb13r[:], start=False, stop=True
                    )

                    # NOTE: only dim 0 of an SBUF AP crosses partitions, so
                    # one DMA per tap t (dst partition stride 9, offset 1+t).
                    nc.vector.tensor_copy(lhsT[0:1, :], ps_b[:])
                    lhsT_g = lhsT[1:KQ, :].rearrange("(n t) d -> n t d", t=9)
                    wgen16_3d = wgen16[:].rearrange("n (t d) -> n t d", t=9)
                    for t in range(9):
                        nc.sync.dma_start(lhsT_g[:, t, :], wgen16_3d[:, t, :])

            # ---------------- main loop: dynamic conv -----------------------
            # One matmul per two output rows: psum[d, 2, w] accumulates all
            # 117 taps + bias in a single pass (contraction dim 118).
            # PSUM -> fp16 SBUF evictions: only DVE and ACT can read PSUM
            # (GPSIMD cannot); load-balance by HW-measured per-instruction
            # cost (single-engine ablations: 73.1/68.8 us per iter for 64
            # evictions -> 1142/1075 ns each).
            ev_load = {"v": 0.0, "a": 0.0}
            EV_COST = {"v": 1142.0, "a": 1075.0}
            # The loop is DMA-paced, so the PE idles ~30% of each group; on
            # HW that de-ramps the tensor-engine clock to the 1.2 GHz
            # p-state, nearly doubling matmul time. Filler matmuls into a
            # never-read scratch psum bank (1-row contraction, so the weight
            # reload is free) bridge the idle gaps and hold the 2.4 GHz
            # clock. They depend only on the scratch bank (PE-internal WAW),
            # so they run exactly when the PE would otherwise stall.
            FILL = 0
            for flag in ab:
                if flag.startswith("fill") and flag[4:].isdigit():
                    FILL = int(flag[4:])
            # "p1": single-bank psum tiles (7 in flight + 1 scratch bank for
            # fillers) with per-pair evictions — finer-grained pipelining.
            # default: four 2-bank psum tiles, 4-row evictions.
            P1 = "p1" in ab
            n_ps = 8 if P1 else 4
            ps_shape = [DO, 2, WW] if P1 else [DO, 4, WW]
            ps_bufs = 7 if P1 else 4
            with (
                tc.tile_pool(name="ostage", bufs=8 if "ost8" in ab else 6) as ostage_pool,
                tc.tile_pool(name="cpsum", bufs=ps_bufs, space="PSUM") as cpsum_pool,
                tc.tile_pool(name="fpsum", bufs=1, space="PSUM") as fpsum_pool,
            ):
                fscr = None
                if FILL and P1:
                    fscr = fpsum_pool.tile([DO, 2, WW], F32)
                for _rep in range(repeat):
                    for grp in range(NGRP):
                        y0 = grp * GRP_ROWS
                        psums = [
                            cpsum_pool.tile(
                                ps_shape, F32, tag="cps", name=f"cps{i}"
                            )
                            for i in range(n_ps)
                        ]
                        if "mm4" in ab and not P1:
                            # one matmul per 2-bank psum tile (free=1024)
                            for i in range(4):
                                r0 = y0 + i * 4
                                nc.tensor.matmul(
                                    psums[i][:],
                                    lhsT[:],
                                    x117[:, r0:r0 + 4, :],
                                    start=True,
                                    stop=True,
                                )
                        else:
                            mm_per = 1 if P1 else 2
                            for i in range(n_ps):
                                for j in range(mm_per):
                                    r0 = y0 + (i * mm_per + j) * 2
                                    nc.tensor.matmul(
                                        psums[i][:, 2 * j:2 * j + 2, :],
                                        lhsT[:],
                                        x117[:, r0:r0 + 2, :],
                                        start=True,
                                        stop=True,
                                    )
                        if fscr is not None:
                            for _f in range(FILL):
                                nc.tensor.matmul(
                                    fscr[:], lhsT[0:1, :], x117[0:1, 0:2, :],
                                    start=True, stop=True,
                                )
                        ost = ostage_pool.tile([DO, GRP_ROWS, WW], F16, tag="ost")
                        if "evsplit" in ab and not P1:
                            # halve psum-recycle latency: both engines evict
                            # halves of the same tile concurrently
                            for i in range(4):
                                lohalf = ost[:, 4 * i:4 * i + 2, :]
                                hihalf = ost[:, 4 * i + 2:4 * i + 4, :]
                                if i % 2 == 0:
                                    nc.vector.tensor_copy(lohalf, psums[i][:, 0:2, :])
                                    nc.scalar.copy(hihalf, psums[i][:, 2:4, :])
                                else:
                                    nc.scalar.copy(lohalf, psums[i][:, 0:2, :])
                                    nc.vector.tensor_copy(hihalf, psums[i][:, 2:4, :])
                            dma_eng = (nc.sync, nc.gpsimd)[grp % 2]
                            if "outslim" in ab:
                                dma_eng.dma_start(
                                    out_ext.ap()[:, y0:y0 + GRP_ROWS, 0:16],
                                    ost[:, :, 0:16],
                                )
                            else:
                                dma_eng.dma_start(
                                    out_ext.ap()[:, y0:y0 + GRP_ROWS, :], ost[:]
                                )
                            continue
                        rows_per = 2 if P1 else 4
                        for i in range(n_ps):
                            if "evdve" in ab:
                                eng = "v"
                            elif "evact" in ab:
                                eng = "a"
                            else:
                                eng = min(
                                    ev_load, key=lambda e: ev_load[e] + EV_COST[e]
                                )
                            ev_load[eng] += EV_COST[eng]
                            dst = ost[:, rows_per * i:rows_per * (i + 1), :]
                            if eng == "v":
                                nc.vector.tensor_copy(dst, psums[i][:])
                            else:
                                nc.scalar.copy(dst, psums[i][:])
                        # trigger from SP/GPSIMD only: a dma_start on an
                        # evicting engine's in-order SEQ would stall its
                        # later evictions while waiting on the other
                        # engine's eviction semaphores.
                        dma_eng = (nc.sync, nc.gpsimd)[grp % 2]
                        if "outslim" in ab:
                            dma_eng.dma_start(
                                out_ext.ap()[:, y0:y0 + GRP_ROWS, 0:16],
                                ost[:, :, 0:16],
                            )
                        elif "dma16" in ab:
                            dma_eng.dma_start(
                                out_ext.ap()[:, y0:y0 + GRP_ROWS, :], ost[:]
                            )
                        elif "dma4" in ab:
                            # one 4-row DMA per eviction: each starts as soon
                            # as its psum tile is staged
                            for h in range(4):
                                eng = (nc.sync, nc.gpsimd)[h % 2]
                                eng.dma_start(
                                    out_ext.ap()[:, y0 + 4 * h:y0 + 4 * (h + 1), :],
                                    ost[:, 4 * h:4 * (h + 1), :],
                                )
                        else:
                            # two 8-row DMAs per group: each starts once its
                            # half of the evictions lands (lower handoff
                            # latency than one 16-row DMA, same bandwidth)
                            for h, eng in ((0, nc.sync), (1, nc.gpsimd)):
                                eng.dma_start(
                                    out_ext.ap()[:, y0 + 8 * h:y0 + 8 * (h + 1), :],
                                    ost[:, 8 * h:8 * (h + 1), :],
                                )
    if not nc.is_finalized():
        nc.finalize()
    return nc


_NC_CACHE = None


def _get_bass():
    global _NC_CACHE
    if _NC_CACHE is None:
        _NC_CACHE = _build_bass()
    return _NC_CACHE


def _prep_in_maps(inputs):
    x16 = np.asarray(inputs["x"], dtype=np.float32).astype(np.float16)
    x = np.zeros((x16.shape[0], NB, HH, WPAD), np.float16)
    x[:, :, :, 1:WW + 1] = x16
    t_emb = np.ascontiguousarray(np.asarray(inputs["t_emb"], dtype=np.float32))
    wv = np.ascontiguousarray(np.asarray(inputs["wv_embs"], dtype=np.float32))
    w1 = np.asarray(inputs["w1"], dtype=np.float32)
    b1 = np.ascontiguousarray(np.asarray(inputs["b1"], dtype=np.float32))
    w2 = np.asarray(inputs["w2"], dtype=np.float32)
    b2 = np.asarray(inputs["b2"], dtype=np.float32)
    wb = np.asarray(inputs["wb"], dtype=np.float32)
    bb = np.asarray(inputs["bb"], dtype=np.float32)

    # permute filter columns: c = d*9 + p  ->  c' = p*128 + d; cast to fp16
    w2p = w2.reshape(4 * DO, DO, 9).transpose(0, 2, 1).reshape(4 * DO, DO * 9)
    w2pp = np.ascontiguousarray(
        w2p.reshape(4, 128, DO * 9).transpose(1, 0, 2)
    ).astype(np.float16)
    b2p = np.ascontiguousarray(b2.reshape(DO, 9).T.reshape(DO * 9)).astype(np.float16)
    w1p = np.ascontiguousarray(
        w1.reshape(8, 128, 4 * DO).transpose(1, 0, 2)
    ).astype(np.float16)
    wbp = np.ascontiguousarray(
        wb.reshape(8, 128, DO).transpose(1, 0, 2)
    ).astype(np.float16)
    bb13 = np.ascontiguousarray(float(NB) * bb).astype(np.float16)
    ones = np.ones((1, HH, WW), np.float16)

    return [
        {
            "x": x[b], "t_emb": t_emb[b], "wv": wv[b],
            "w1p": w1p, "b1": b1, "w2pp": w2pp, "b2p": b2p,
            "wbp": wbp, "bb13": bb13, "ones": ones,
        }
        for b in range(NCORES)
    ]


def kernel(**inputs) -> np.ndarray:
    nc = _get_bass()
    in_maps = _prep_in_maps(inputs)
    res = run_bass_kernel_spmd(nc, in_maps, list(range(NCORES)))
    out16 = np.stack([res.results[b]["out"] for b in range(NCORES)], axis=0)
    return out16.astype(np.float32)


if __name__ == "__main__":
    rng = np.random.default_rng(0)
    demo = {
        "x": rng.standard_normal((NCORES, NB, HH, WW), dtype=np.float32),
        "t_emb": rng.standard_normal((NCORES, DE), dtype=np.float32),
        "wv_embs": rng.standard_normal((NCORES, NB, DE), dtype=np.float32),
        "w1": rng.standard_normal((DE, 4 * DO), dtype=np.float32) * 0.02,
        "b1": np.zeros(4 * DO, np.float32),
        "w2": rng.standard_normal((4 * DO, DO * 9), dtype=np.float32) * 0.02,
        "b2": np.zeros(DO * 9, np.float32),
        "wb": rng.standard_normal((DE, DO), dtype=np.float32) * 0.02,
        "bb": np.zeros(DO, np.float32),
    }
    out = kernel(**demo)
    print("out", out.shape, out.dtype, float(np.abs(out).mean()))
